# revision 24
# baseline (speedup 1.0000x reference)
"""Block-sparse attention (mean-similarity top-k) on 8 TRN2 NeuronCores — v2.

Sharding: 16 heads tensor-parallel across 8 cores (2 heads/core); proj
row-sharded with bf16 partials summed on host.

v2 structure (vs v1):
  - host uploads x as bf16 (halves input DMA) plus exact f32 block sums
    (xm), so selection runs first and needs no on-device reductions/casts
  - phase order: selection -> K -> V -> per-512-token chunk {q, 4x
    (scores+exp+o)} with proj trailing one chunk; ACT does exp almost
    exclusively and is busy from ~25us on instead of ~65us
  - V produced d-major then PE-transposed to token-major (64 fewer
    LDW+MM pairs than direct token-major)
  - o normalized then PE-transposed on-chip; the HBM obounce round trip
    and DMA transposes are gone
  - proj bias folded into the DVE psum evacuation (bf16 out, halves the
    output DMA); partials summed on host in f32
"""

import os
import sys

for _p in ("/opt/trn_rl_repo", "/root/.axon_site/_ro/trn_rl_repo"):
    if os.path.isdir(_p) and _p not in sys.path:
        sys.path.insert(0, _p)

import numpy as np
import ml_dtypes

import concourse.bass as bass
import concourse.bacc as bacc
import concourse.tile as tile
import concourse.mybir as mybir
from concourse.bass_utils import run_bass_kernel_spmd
from concourse.library_config import ap_gather as ap_gather_lib

# problem constants
N = 4096          # sequence length
C = 1024          # model dim
H = 16            # heads
D = 64            # head dim
BLK = 128         # block size
NB = N // BLK     # 32 blocks
TOPK = 16         # int(0.5 * NB)
NCORES = 8
HPC = H // NCORES  # 2 heads per core
KC = C // 128      # 8 contraction tiles
SCALE = D ** -0.5  # 0.125
CHQ = 4            # query blocks per chunk (512 tokens)
NCH = NB // CHQ    # 8 chunks

F32 = mybir.dt.float32
BF16 = mybir.dt.bfloat16
I16 = mybir.dt.int16
U32 = mybir.dt.uint32

_CACHE = {}


def _build():
    nc = bacc.Bacc("TRN2", target_bir_lowering=False, debug=False,
                   num_devices=NCORES)

    xbf_ext = nc.dram_tensor("xbf", [C, N], BF16, kind="ExternalInput")
    xm_ext = nc.dram_tensor("xm", [C, NB], F32, kind="ExternalInput")
    wqksel_ext = nc.dram_tensor("wqksel", [C, 256], F32, kind="ExternalInput")
    wqkv_ext = nc.dram_tensor("wqkvT", [C, 384], BF16, kind="ExternalInput")
    projW_ext = nc.dram_tensor("projWT", [128, C], BF16, kind="ExternalInput")
    projb_ext = nc.dram_tensor("projb", [128, KC], F32, kind="ExternalInput")
    ident64_ext = nc.dram_tensor("ident64", [64, 64], F32, kind="ExternalInput")
    identb_ext = nc.dram_tensor("identb", [128, 128], BF16, kind="ExternalInput")
    erep_ext = nc.dram_tensor("erep", [16, 128], F32, kind="ExternalInput")
    out_ext = nc.dram_tensor("out", [C, N], BF16, kind="ExternalOutput")

    with tile.TileContext(nc) as tc:
        nc.gpsimd.load_library(ap_gather_lib)

        with tc.tile_pool(name="persist", bufs=1) as pp:
            # ---- small inputs first (selection path) ----
            wqksel = pp.tile([128, KC, 256], F32)
            nc.sync.dma_start(
                wqksel[:], wqksel_ext.ap().rearrange("(a p) m -> p a m", p=128))
            xm = pp.tile([128, KC, NB], F32)
            nc.sync.dma_start(
                xm[:], xm_ext.ap().rearrange("(a p) m -> p a m", p=128))
            erep_sb = pp.tile([16, 128], F32)
            nc.sync.dma_start(erep_sb[:], erep_ext.ap())
            id64 = pp.tile([64, 64], F32)
            nc.sync.dma_start(id64[:], ident64_ext.ap())
            identb = pp.tile([128, 128], BF16)
            nc.sync.dma_start(identb[:], identb_ext.ap())
            wqkv_bf = pp.tile([128, KC, 384], BF16)
            nc.sync.dma_start(
                wqkv_bf[:], wqkv_ext.ap().rearrange("(a p) m -> p a m", p=128))
            projW_bf = pp.tile([128, C], BF16)
            nc.sync.dma_start(projW_bf[:], projW_ext.ap())
            projb_sb = pp.tile([128, KC], F32)
            nc.sync.dma_start(projb_sb[:], projb_ext.ap())

            # preload the Exp activation table off the critical path
            dummy_exp = pp.tile([128, 8], BF16)
            nc.scalar.activation(dummy_exp[:], xm[:, 0, 0:8],
                                 mybir.ActivationFunctionType.Exp, scale=1.0)

            # ---- x (bf16), chunked for DMA/compute pipelining ----
            xbf = pp.tile([128, KC, N], BF16)
            xsrc = xbf_ext.ap().rearrange("(a p) n -> p a n", p=128)
            for nch in range(NCH):
                lo, hi = nch * 512, (nch + 1) * 512
                nc.sync.dma_start(xbf[:, :, lo:hi], xsrc[:, :, lo:hi])

            # ---- persistent activations ----
            qT = pp.tile([128, N], BF16)
            kT = pp.tile([128, NB, BLK], BF16)
            v0 = pp.tile([128, NB, 66], BF16)
            v1 = pp.tile([128, NB, 66], BF16)
            nc.vector.memset(v0[:, :, 64:66], 0.0)
            nc.vector.memset(v1[:, :, 64:66], 0.0)
            nc.vector.memset(v0[:, :, 64:65], 1.0)
            nc.vector.memset(v1[:, :, 64:65], 1.0)
            kidx = pp.tile([128, NB], I16)
            vidx0 = pp.tile([128, NB], I16)
            vidx1 = pp.tile([128, NB], I16)

            # single shared PSUM pool set (8 banks total):
            #   s: 3 x [128,1024] f32 = 6 banks (scores; 3 slots so the PE
            #      runs a full tile ahead of the exp drain)
            #   q: 1 x [128,512] f32  = 1 bank (selection, qkv, q chunks,
            #      proj, transposes)
            #   o: 1 x [128,130] f32  = 1 bank (both heads' o accumulation)
            with tc.tile_pool(name="ps", bufs=1, space="PSUM") as qp, \
                 tc.tile_pool(name="ops", bufs=1, space="PSUM") as opp, \
                 tc.tile_pool(name="sps", bufs=3, space="PSUM") as spp, \
                 tc.tile_pool(name="selsb", bufs=2) as sb, \
                 tc.tile_pool(name="vts", bufs=2) as vtp, \
                 tc.tile_pool(name="gather", bufs=6) as gp, \
                 tc.tile_pool(name="escore", bufs=16) as ep, \
                 tc.tile_pool(name="otp", bufs=5) as otp, \
                 tc.tile_pool(name="prout", bufs=6) as pr, \
                 tc.tile_pool(name="osb", bufs=8) as ob:

                # ---- selection (f32): block sims + top-16 per head ----
                qm_ps = qp.tile([128, NB], F32, tag="q")
                for kc in range(KC):
                    nc.tensor.matmul(qm_ps[:], lhsT=wqksel[:, kc, 0:128],
                                     rhs=xm[:, kc, :], start=(kc == 0),
                                     stop=(kc == KC - 1))
                km_ps = qp.tile([128, NB], F32, tag="q")
                for kc in range(KC):
                    nc.tensor.matmul(km_ps[:], lhsT=wqksel[:, kc, 128:256],
                                     rhs=xm[:, kc, :], start=(kc == 0),
                                     stop=(kc == KC - 1))
                qm_sb = sb.tile([128, NB], F32, tag="qm")
                km_sb = sb.tile([128, NB], F32, tag="km")
                nc.scalar.copy(qm_sb[:], qm_ps[:])
                nc.scalar.copy(km_sb[:], km_ps[:])

                sim_ps = qp.tile([64, NB], F32, tag="q")
                for h in range(HPC):
                    nc.tensor.matmul(sim_ps[h * 32:(h + 1) * 32, :],
                                     lhsT=qm_sb[h * 64:(h + 1) * 64, :],
                                     rhs=km_sb[h * 64:(h + 1) * 64, :],
                                     start=True, stop=True)
                sim2 = sb.tile([64, NB], F32, tag="sim2")
                nc.vector.tensor_copy(sim2[:], sim_ps[:])

                vals0 = sb.tile([64, 8], F32, tag="v0")
                idx0 = sb.tile([64, 8], U32, tag="i0")
                pun = sb.tile([64, NB], F32, tag="pun")
                vals1 = sb.tile([64, 8], F32, tag="v1")
                idx1 = sb.tile([64, 8], U32, tag="i1")
                nc.vector.max(vals0[:], sim2[:])
                nc.vector.max_index(idx0[:], vals0[:], sim2[:])
                nc.vector.match_replace(out=pun[:], in_to_replace=vals0[:],
                                        in_values=sim2[:], imm_value=-1e30)
                nc.vector.max(vals1[:], pun[:])
                nc.vector.max_index(idx1[:], vals1[:], pun[:])

                idxf = sb.tile([64, TOPK], F32, tag="idxf")
                nc.vector.tensor_copy(idxf[:, 0:8], idx0[:])
                nc.vector.tensor_copy(idxf[:, 8:16], idx1[:])

                selT_ps = qp.tile([TOPK, 64], F32, tag="q")
                nc.tensor.transpose(selT_ps[:], idxf[:], id64[:])
                selT = sb.tile([TOPK, 64], F32, tag="selTsb")
                nc.vector.tensor_copy(selT[:], selT_ps[:])

                # replicate selT rows to all 16-partition groups: rep[m, n] =
                # selT[m % 16, n]
                rep_ps = qp.tile([128, 64], F32, tag="q")
                nc.tensor.matmul(rep_ps[:], lhsT=erep_sb[:], rhs=selT[:],
                                 start=True, stop=True)
                nc.vector.tensor_copy(kidx[0:64, :], rep_ps[0:64, 0:32])
                nc.vector.tensor_copy(kidx[64:128, :], rep_ps[64:128, 32:64])
                nc.vector.tensor_copy(vidx0[:], rep_ps[:, 0:32])
                nc.vector.tensor_copy(vidx1[:], rep_ps[:, 32:64])

                # ---- K phase (DMA-paced); evac on ACT ----
                for nch in range(NCH):
                    lo, hi = nch * 512, (nch + 1) * 512
                    ps = qp.tile([128, 512], F32, tag="q", name=f"kps_{nch}")
                    for kc in range(KC):
                        nc.tensor.matmul(ps[:], lhsT=wqkv_bf[:, kc, 128:256],
                                         rhs=xbf[:, kc, lo:hi],
                                         start=(kc == 0), stop=(kc == KC - 1))
                    nc.scalar.copy(
                        kT[:].rearrange("p a b -> p (a b)")[:, lo:hi], ps[:])

                def emit_v(nch):
                    lo, hi = nch * 512, (nch + 1) * 512
                    psv = opp.tile([128, 512], F32, tag="o", name=f"vps_{nch}")
                    for kc in range(KC):
                        nc.tensor.matmul(psv[:], lhsT=wqkv_bf[:, kc, 256:384],
                                         rhs=xbf[:, kc, lo:hi],
                                         start=(kc == 0), stop=(kc == KC - 1))
                    vts = vtp.tile([128, 512], BF16, tag="vt", name=f"vt_{nch}")
                    nc.vector.tensor_copy(vts[:], psv[:])
                    for nt in range(4):
                        blk = 4 * nch + nt
                        # the scores slots are idle during this phase
                        tp = spp.tile([128, 128], BF16, tag="s",
                                      name=f"vtr_{blk}")
                        nc.tensor.transpose(
                            tp[:], vts[:, nt * 128:(nt + 1) * 128], identb[:])
                        nc.vector.tensor_copy(v0[:, blk, 0:64], tp[:, 0:64])
                        nc.scalar.copy(v1[:, blk, 0:64], tp[:, 64:128])

                # ---- main loop ----
                state = {}
                chunk_ot = {}
                kgs = {}

                def emit_q(cc):
                    lo, hi = cc * 512, (cc + 1) * 512
                    ps = qp.tile([128, 512], F32, tag="q", name=f"qps_{cc}")
                    for kc in range(KC):
                        nc.tensor.matmul(ps[:], lhsT=wqkv_bf[:, kc, 0:128],
                                         rhs=xbf[:, kc, lo:hi],
                                         start=(kc == 0), stop=(kc == KC - 1))
                    nc.vector.tensor_copy(qT[:, lo:hi], ps[:])

                podma_pending = []

                def emit_proj_m(cc, m, pool=None):
                    # one projection m-tile of chunk cc; the output DMA is
                    # deferred an iteration so the in-order Sync queue never
                    # sits on an unfinished evacuation
                    ot = chunk_ot[cc]
                    pool, tag = (qp, "q") if pool is None else pool
                    pj = pool.tile([128, 512], F32, tag=tag,
                                   name=f"pj_{cc}_{m}")
                    nc.tensor.matmul(pj[:],
                                     lhsT=projW_bf[:, m * 128:(m + 1) * 128],
                                     rhs=ot[:], start=True, stop=True)
                    po = pr.tile([128, 512], BF16, tag="po",
                                 name=f"po_{cc}_{m}")
                    nc.vector.tensor_scalar(po[:], pj[:],
                                            projb_sb[:, m:m + 1], None,
                                            op0=mybir.AluOpType.add)
                    podma_pending.append((cc, m, po))

                def flush_podma():
                    while podma_pending:
                        cc, m, po = podma_pending.pop()
                        nc.sync.dma_start(
                            out_ext.ap()[m * 128:(m + 1) * 128,
                                         cc * 512:(cc + 1) * 512],
                            po[:])

                def emit_gk(qb):
                    kg = gp.tile([128, TOPK, BLK], BF16, tag="kg",
                                 name=f"kg_{qb}")
                    nc.gpsimd.ap_gather(kg[:], kT[:], kidx[:, qb:qb + 1],
                                        channels=128, num_elems=NB, d=BLK,
                                        num_idxs=TOPK)
                    kgs[qb] = kg

                def emit_vg(qb):
                    if qb not in state:
                        return
                    vg0 = gp.tile([128, TOPK, 66], BF16, tag="vg0",
                                  name=f"vg0_{qb}")
                    nc.gpsimd.ap_gather(vg0[:], v0[:], vidx0[:, qb:qb + 1],
                                        channels=128, num_elems=NB, d=66,
                                        num_idxs=TOPK)
                    vg1 = gp.tile([128, TOPK, 66], BF16, tag="vg1",
                                  name=f"vg1_{qb}")
                    nc.gpsimd.ap_gather(vg1[:], v1[:], vidx1[:, qb:qb + 1],
                                        channels=128, num_elems=NB, d=66,
                                        num_idxs=TOPK)
                    state[qb][1:3] = [vg0, vg1]

                def emit_smm(qb):
                    kg = kgs.pop(qb)
                    qcol = slice(qb * BLK, (qb + 1) * BLK)
                    etiles = [[None, None], [None, None]]
                    for half in range(2):
                        s0 = spp.tile([128, 1024], F32, tag="s",
                                      name=f"s0_{qb}_{half}")
                        s1 = spp.tile([128, 1024], F32, tag="s",
                                      name=f"s1_{qb}_{half}")
                        for jj in range(8):
                            j = half * 8 + jj
                            nc.tensor.matmul(s0[:, jj * 128:(jj + 1) * 128],
                                             lhsT=kg[0:64, j, :],
                                             rhs=qT[0:64, qcol],
                                             start=True, stop=True)
                            nc.tensor.matmul(s1[:, jj * 128:(jj + 1) * 128],
                                             lhsT=kg[64:128, j, :],
                                             rhs=qT[64:128, qcol],
                                             start=True, stop=True)
                        e0 = ep.tile([128, 1024], BF16, tag="e",
                                     name=f"e0_{qb}_{half}")
                        e1 = ep.tile([128, 1024], BF16, tag="e",
                                     name=f"e1_{qb}_{half}")
                        nc.scalar.activation(e0[:], s0[:],
                                             mybir.ActivationFunctionType.Exp,
                                             scale=SCALE)
                        nc.scalar.activation(e1[:], s1[:],
                                             mybir.ActivationFunctionType.Exp,
                                             scale=SCALE)
                        etiles[0][half] = e0
                        etiles[1][half] = e1
                    onorm = ob.tile([128, 2 * D], BF16, tag="onorm",
                                    name=f"on_{qb}")
                    state[qb] = [etiles, None, None, onorm]

                onorms = {}

                def emit_o(qb):
                    if qb not in state:
                        return
                    etiles, vg0, vg1, onorm = state[qb]
                    o_ps = opp.tile([128, 2, D + 1], F32, tag="o",
                                    name=f"o_{qb}")
                    for h in range(2):
                        vg = vg0 if h == 0 else vg1
                        for j in range(TOPK):
                            nc.tensor.matmul(
                                o_ps[:, h, :],
                                lhsT=etiles[h][j // 8][:, (j % 8) * 128:(j % 8 + 1) * 128],
                                rhs=vg[:, j, 0:D + 1],
                                start=(j == 0), stop=(j == TOPK - 1))
                    rec = ob.tile([128, 2], F32, tag="rec", name=f"r_{qb}")
                    nc.vector.reciprocal(rec[:], o_ps[:, :, D])
                    for h in range(2):
                        nc.vector.tensor_scalar(onorm[:, h * D:(h + 1) * D],
                                                o_ps[:, h, 0:D],
                                                rec[:, h:h + 1], None,
                                                op0=mybir.AluOpType.mult)
                    state.pop(qb)
                    onorms[qb] = onorm

                def emit_oT(qb):
                    # o -> o^T via SBUF->SBUF DMA transpose: costs no PE
                    # pair, no psum slot, and no DVE time
                    if qb not in onorms:
                        return
                    onorm = onorms.pop(qb)
                    cc = qb // CHQ
                    if qb % CHQ == 0:
                        chunk_ot[cc] = otp.tile([128, 512], BF16, tag="ot",
                                                name=f"ot_{cc}")
                    nc.sync.dma_start_transpose(
                        chunk_ot[cc][:, (qb % CHQ) * 128:(qb % CHQ + 1) * 128],
                        onorm[:])

                # proj m-tiles of chunk cc are spread across the following
                # chunk's iterations (2 per qb), keyed by the oT-complete qb
                proj_sched = {}

                def schedule_proj(cc):
                    base = CHQ * (cc + 1) + 2
                    for m in range(KC):
                        proj_sched.setdefault(base + m // 2, []).append((cc, m))

                # prologue: q + scores for the first two blocks flow while
                # the V phase computes, so the exp stream starts ~15us early
                emit_q(0)
                emit_q(1)
                emit_gk(0)
                emit_gk(1)
                emit_gk(2)
                emit_smm(0)
                emit_smm(1)
                for nch in range(NCH):
                    emit_v(nch)
                emit_vg(0)
                emit_vg(1)
                # steady state, per iteration qb: o(qb-1) -> oT -> proj a ->
                # k-gather(qb+2) -> scores(qb+1) -> proj b -> v-gather(qb+1);
                # scores go last because they self-throttle against the exp
                # drain (3 psum slots) and nothing may queue behind them
                for qb in range(1, NB):
                    cc = qb // CHQ
                    if qb % CHQ == 1 and cc + 1 < NCH:
                        emit_q(cc + 1)
                    emit_oT(qb - 2)
                    flush_podma()
                    emit_o(qb - 1)
                    pops = proj_sched.pop(qb, ())
                    if len(pops) > 0:
                        emit_proj_m(*pops[0])
                    if qb + 2 < NB:
                        emit_gk(qb + 2)
                    if qb + 1 < NB:
                        emit_smm(qb + 1)
                    if len(pops) > 1:
                        emit_proj_m(*pops[1])
                    if qb + 1 < NB:
                        emit_vg(qb + 1)
                    if qb >= 2 and (qb - 2) % CHQ == CHQ - 1:
                        schedule_proj((qb - 2) // CHQ)
                # drain
                emit_o(NB - 1)
                emit_oT(NB - 2)
                emit_oT(NB - 1)
                drain_idx = 0
                for qb in sorted(proj_sched):
                    for cm in proj_sched[qb]:
                        emit_proj_m(*cm, pool=((qp, "q"), (opp, "o"))[drain_idx % 2])
                        drain_idx += 1
                proj_sched.clear()
                for m in range(KC):
                    emit_proj_m(NCH - 1, m, pool=((qp, "q"), (opp, "o"))[drain_idx % 2])
                    drain_idx += 1
                flush_podma()

    nc.compile()
    return nc


def _prep_inputs(x, qkv_w, proj_w, proj_b):
    x = np.asarray(x, dtype=np.float32)
    qkv_w = np.asarray(qkv_w, dtype=np.float32)
    proj_w = np.asarray(proj_w, dtype=np.float32)
    proj_b = np.asarray(proj_b, dtype=np.float32)

    xT = np.ascontiguousarray(x[0].T)                         # [C, N]
    xbf = xT.astype(ml_dtypes.bfloat16)
    # exact f32 block sums: selection-equivalent to reference block means
    xm = np.ascontiguousarray(
        x[0].reshape(NB, BLK, C).sum(axis=1, dtype=np.float64).T
    ).astype(np.float32)                                      # [C, NB]
    ident64 = np.eye(64, dtype=np.float32)
    identb = np.eye(128, dtype=ml_dtypes.bfloat16)
    erep = (np.arange(128)[None, :] % 16 == np.arange(16)[:, None]).astype(np.float32)
    zero_b = np.zeros((128, KC), dtype=np.float32)
    in_maps = []
    for i in range(NCORES):
        h0 = HPC * i
        rows = []
        for part in range(3):                                 # q, k, v rows
            base = part * C + h0 * D
            rows.append(qkv_w[base:base + HPC * D, :])
        wqkv = np.concatenate(rows, axis=0)                   # [384, C]
        wqkvT = np.ascontiguousarray(wqkv.T)                  # [C, 384]
        cslice = slice(i * 2 * D, (i + 1) * 2 * D)
        in_maps.append({
            "xbf": xbf,
            "xm": xm,
            "wqksel": np.ascontiguousarray(wqkvT[:, 0:256]).astype(np.float32),
            "wqkvT": wqkvT.astype(ml_dtypes.bfloat16),
            "projWT": np.ascontiguousarray(proj_w[:, cslice].T).astype(ml_dtypes.bfloat16),
            "projb": (np.ascontiguousarray(proj_b.reshape(KC, 128).T)
                      if i == 0 else zero_b),
            "ident64": ident64,
            "identb": identb,
            "erep": erep,
        })
    return in_maps


def kernel(x, qkv_w, proj_w, proj_b, _trace=False):
    if "nc" not in _CACHE:
        _CACHE["nc"] = _build()
    nc = _CACHE["nc"]
    in_maps = _prep_inputs(x, qkv_w, proj_w, proj_b)
    res = run_bass_kernel_spmd(nc, in_maps, core_ids=list(range(NCORES)),
                               trace=_trace)
    outT = res.results[0]["out"].astype(np.float32)
    for i in range(1, NCORES):
        outT += res.results[i]["out"].astype(np.float32)
    out = np.ascontiguousarray(outT.T).reshape(1, N, C).astype(np.float32)
    if _trace:
        _CACHE["last_exec_time_ns"] = res.exec_time_ns
        _CACHE["last_results"] = res
    return out


# revision 25
# speedup vs baseline: 1.0195x; 1.0195x over previous
"""Block-sparse attention (mean-similarity top-k) on 8 TRN2 NeuronCores — v2.

Sharding: 16 heads tensor-parallel across 8 cores (2 heads/core); proj
row-sharded with bf16 partials summed on host.

v2 structure (vs v1):
  - host uploads x as bf16 (halves input DMA) plus exact f32 block sums
    (xm), so selection runs first and needs no on-device reductions/casts
  - phase order: selection -> K -> V -> per-512-token chunk {q, 4x
    (scores+exp+o)} with proj trailing one chunk; ACT does exp almost
    exclusively and is busy from ~25us on instead of ~65us
  - V produced d-major then PE-transposed to token-major (64 fewer
    LDW+MM pairs than direct token-major)
  - o normalized then PE-transposed on-chip; the HBM obounce round trip
    and DMA transposes are gone
  - proj bias folded into the DVE psum evacuation (bf16 out, halves the
    output DMA); partials summed on host in f32
"""

import os
import sys

for _p in ("/opt/trn_rl_repo", "/root/.axon_site/_ro/trn_rl_repo"):
    if os.path.isdir(_p) and _p not in sys.path:
        sys.path.insert(0, _p)

import numpy as np
import ml_dtypes

import concourse.bass as bass
import concourse.bacc as bacc
import concourse.tile as tile
import concourse.mybir as mybir
from concourse.bass_utils import run_bass_kernel_spmd
from concourse.library_config import ap_gather as ap_gather_lib

# problem constants
N = 4096          # sequence length
C = 1024          # model dim
H = 16            # heads
D = 64            # head dim
BLK = 128         # block size
NB = N // BLK     # 32 blocks
TOPK = 16         # int(0.5 * NB)
NCORES = 8
HPC = H // NCORES  # 2 heads per core
KC = C // 128      # 8 contraction tiles
SCALE = D ** -0.5  # 0.125
CHQ = 4            # query blocks per chunk (512 tokens)
NCH = NB // CHQ    # 8 chunks

F32 = mybir.dt.float32
BF16 = mybir.dt.bfloat16
I16 = mybir.dt.int16
U32 = mybir.dt.uint32

_CACHE = {}


def _build():
    nc = bacc.Bacc("TRN2", target_bir_lowering=False, debug=False,
                   num_devices=NCORES)

    xbf_ext = nc.dram_tensor("xbf", [C, N], BF16, kind="ExternalInput")
    xm_ext = nc.dram_tensor("xm", [C, NB], F32, kind="ExternalInput")
    wqksel_ext = nc.dram_tensor("wqksel", [C, 256], F32, kind="ExternalInput")
    wqkv_ext = nc.dram_tensor("wqkvT", [C, 384], BF16, kind="ExternalInput")
    projW_ext = nc.dram_tensor("projWT", [128, C], BF16, kind="ExternalInput")
    projb_ext = nc.dram_tensor("projb", [128, KC], F32, kind="ExternalInput")
    ident64_ext = nc.dram_tensor("ident64", [64, 64], F32, kind="ExternalInput")
    identb_ext = nc.dram_tensor("identb", [128, 128], BF16, kind="ExternalInput")
    erep_ext = nc.dram_tensor("erep", [16, 128], F32, kind="ExternalInput")
    out_ext = nc.dram_tensor("out", [C, N], BF16, kind="ExternalOutput")

    with tile.TileContext(nc) as tc:
        nc.gpsimd.load_library(ap_gather_lib)

        with tc.tile_pool(name="persist", bufs=1) as pp:
            # ---- small inputs first (selection path) ----
            wqksel = pp.tile([128, KC, 256], F32)
            nc.sync.dma_start(
                wqksel[:], wqksel_ext.ap().rearrange("(a p) m -> p a m", p=128))
            xm = pp.tile([128, KC, NB], F32)
            nc.sync.dma_start(
                xm[:], xm_ext.ap().rearrange("(a p) m -> p a m", p=128))
            erep_sb = pp.tile([16, 128], F32)
            nc.sync.dma_start(erep_sb[:], erep_ext.ap())
            id64 = pp.tile([64, 64], F32)
            nc.sync.dma_start(id64[:], ident64_ext.ap())
            identb = pp.tile([128, 128], BF16)
            nc.sync.dma_start(identb[:], identb_ext.ap())
            wqkv_bf = pp.tile([128, KC, 384], BF16)
            nc.sync.dma_start(
                wqkv_bf[:], wqkv_ext.ap().rearrange("(a p) m -> p a m", p=128))
            projW_bf = pp.tile([128, C], BF16)
            nc.sync.dma_start(projW_bf[:], projW_ext.ap())
            projb_sb = pp.tile([128, KC], F32)
            nc.sync.dma_start(projb_sb[:], projb_ext.ap())

            # preload the Exp activation table off the critical path
            dummy_exp = pp.tile([128, 8], BF16)
            nc.scalar.activation(dummy_exp[:], xm[:, 0, 0:8],
                                 mybir.ActivationFunctionType.Exp, scale=1.0)

            # ---- x (bf16), chunked for DMA/compute pipelining ----
            xbf = pp.tile([128, KC, N], BF16)
            xsrc = xbf_ext.ap().rearrange("(a p) n -> p a n", p=128)
            for nch in range(NCH):
                lo, hi = nch * 512, (nch + 1) * 512
                nc.sync.dma_start(xbf[:, :, lo:hi], xsrc[:, :, lo:hi])

            # ---- persistent activations ----
            qT = pp.tile([128, N], BF16)
            kT = pp.tile([128, NB, BLK], BF16)
            v0 = pp.tile([128, NB, 66], BF16)
            v1 = pp.tile([128, NB, 66], BF16)
            nc.vector.memset(v0[:, :, 64:66], 0.0)
            nc.vector.memset(v1[:, :, 64:66], 0.0)
            nc.vector.memset(v0[:, :, 64:65], 1.0)
            nc.vector.memset(v1[:, :, 64:65], 1.0)
            kidx = pp.tile([128, NB], I16)
            vidx0 = pp.tile([128, NB], I16)
            vidx1 = pp.tile([128, NB], I16)

            # single shared PSUM pool set (8 banks total):
            #   s: 3 x [128,1024] f32 = 6 banks (scores; 3 slots so the PE
            #      runs a full tile ahead of the exp drain)
            #   q: 1 x [128,512] f32  = 1 bank (selection, qkv, q chunks,
            #      proj, transposes)
            #   o: 1 x [128,130] f32  = 1 bank (both heads' o accumulation)
            with tc.tile_pool(name="ps", bufs=1, space="PSUM") as qp, \
                 tc.tile_pool(name="ops", bufs=1, space="PSUM") as opp, \
                 tc.tile_pool(name="sps", bufs=3, space="PSUM") as spp, \
                 tc.tile_pool(name="selsb", bufs=2) as sb, \
                 tc.tile_pool(name="vts", bufs=2) as vtp, \
                 tc.tile_pool(name="gather", bufs=6) as gp, \
                 tc.tile_pool(name="escore", bufs=16) as ep, \
                 tc.tile_pool(name="otp", bufs=5) as otp, \
                 tc.tile_pool(name="prout", bufs=6) as pr, \
                 tc.tile_pool(name="osb", bufs=8) as ob:

                # ---- selection (f32): block sims + top-16 per head ----
                qm_ps = qp.tile([128, NB], F32, tag="q")
                for kc in range(KC):
                    nc.tensor.matmul(qm_ps[:], lhsT=wqksel[:, kc, 0:128],
                                     rhs=xm[:, kc, :], start=(kc == 0),
                                     stop=(kc == KC - 1))
                km_ps = qp.tile([128, NB], F32, tag="q")
                for kc in range(KC):
                    nc.tensor.matmul(km_ps[:], lhsT=wqksel[:, kc, 128:256],
                                     rhs=xm[:, kc, :], start=(kc == 0),
                                     stop=(kc == KC - 1))
                qm_sb = sb.tile([128, NB], F32, tag="qm")
                km_sb = sb.tile([128, NB], F32, tag="km")
                nc.scalar.copy(qm_sb[:], qm_ps[:])
                nc.scalar.copy(km_sb[:], km_ps[:])

                sim_ps = qp.tile([64, NB], F32, tag="q")
                for h in range(HPC):
                    nc.tensor.matmul(sim_ps[h * 32:(h + 1) * 32, :],
                                     lhsT=qm_sb[h * 64:(h + 1) * 64, :],
                                     rhs=km_sb[h * 64:(h + 1) * 64, :],
                                     start=True, stop=True)
                sim2 = sb.tile([64, NB], F32, tag="sim2")
                nc.vector.tensor_copy(sim2[:], sim_ps[:])

                vals0 = sb.tile([64, 8], F32, tag="v0")
                idx0 = sb.tile([64, 8], U32, tag="i0")
                pun = sb.tile([64, NB], F32, tag="pun")
                vals1 = sb.tile([64, 8], F32, tag="v1")
                idx1 = sb.tile([64, 8], U32, tag="i1")
                nc.vector.max(vals0[:], sim2[:])
                nc.vector.max_index(idx0[:], vals0[:], sim2[:])
                nc.vector.match_replace(out=pun[:], in_to_replace=vals0[:],
                                        in_values=sim2[:], imm_value=-1e30)
                nc.vector.max(vals1[:], pun[:])
                nc.vector.max_index(idx1[:], vals1[:], pun[:])

                idxf = sb.tile([64, TOPK], F32, tag="idxf")
                nc.vector.tensor_copy(idxf[:, 0:8], idx0[:])
                nc.vector.tensor_copy(idxf[:, 8:16], idx1[:])

                selT_ps = qp.tile([TOPK, 64], F32, tag="q")
                nc.tensor.transpose(selT_ps[:], idxf[:], id64[:])
                selT = sb.tile([TOPK, 64], F32, tag="selTsb")
                nc.vector.tensor_copy(selT[:], selT_ps[:])

                # replicate selT rows to all 16-partition groups: rep[m, n] =
                # selT[m % 16, n]
                rep_ps = qp.tile([128, 64], F32, tag="q")
                nc.tensor.matmul(rep_ps[:], lhsT=erep_sb[:], rhs=selT[:],
                                 start=True, stop=True)
                nc.vector.tensor_copy(kidx[0:64, :], rep_ps[0:64, 0:32])
                nc.vector.tensor_copy(kidx[64:128, :], rep_ps[64:128, 32:64])
                nc.vector.tensor_copy(vidx0[:], rep_ps[:, 0:32])
                nc.vector.tensor_copy(vidx1[:], rep_ps[:, 32:64])

                # ---- K phase (DMA-paced); evac on ACT ----
                for nch in range(NCH):
                    lo, hi = nch * 512, (nch + 1) * 512
                    ps = qp.tile([128, 512], F32, tag="q", name=f"kps_{nch}")
                    for kc in range(KC):
                        nc.tensor.matmul(ps[:], lhsT=wqkv_bf[:, kc, 128:256],
                                         rhs=xbf[:, kc, lo:hi],
                                         start=(kc == 0), stop=(kc == KC - 1))
                    nc.scalar.copy(
                        kT[:].rearrange("p a b -> p (a b)")[:, lo:hi], ps[:])

                def emit_v(nch):
                    lo, hi = nch * 512, (nch + 1) * 512
                    psv = opp.tile([128, 512], F32, tag="o", name=f"vps_{nch}")
                    for kc in range(KC):
                        nc.tensor.matmul(psv[:], lhsT=wqkv_bf[:, kc, 256:384],
                                         rhs=xbf[:, kc, lo:hi],
                                         start=(kc == 0), stop=(kc == KC - 1))
                    vts = vtp.tile([128, 512], BF16, tag="vt", name=f"vt_{nch}")
                    nc.vector.tensor_copy(vts[:], psv[:])
                    for nt in range(4):
                        blk = 4 * nch + nt
                        # the scores slots are idle during this phase
                        tp = spp.tile([128, 128], BF16, tag="s",
                                      name=f"vtr_{blk}")
                        nc.tensor.transpose(
                            tp[:], vts[:, nt * 128:(nt + 1) * 128], identb[:])
                        nc.vector.tensor_copy(v0[:, blk, 0:64], tp[:, 0:64])
                        nc.scalar.copy(v1[:, blk, 0:64], tp[:, 64:128])

                # ---- main loop ----
                state = {}
                chunk_ot = {}
                kgs = {}

                def emit_q(cc):
                    lo, hi = cc * 512, (cc + 1) * 512
                    ps = qp.tile([128, 512], F32, tag="q", name=f"qps_{cc}")
                    for kc in range(KC):
                        nc.tensor.matmul(ps[:], lhsT=wqkv_bf[:, kc, 0:128],
                                         rhs=xbf[:, kc, lo:hi],
                                         start=(kc == 0), stop=(kc == KC - 1))
                    nc.vector.tensor_copy(qT[:, lo:hi], ps[:])

                podma_pending = []

                def emit_proj_m(cc, m, pool=None):
                    # one projection m-tile of chunk cc; the output DMA is
                    # deferred an iteration so the in-order Sync queue never
                    # sits on an unfinished evacuation
                    ot = chunk_ot[cc]
                    pool, tag = (qp, "q") if pool is None else pool
                    pj = pool.tile([128, 512], F32, tag=tag,
                                   name=f"pj_{cc}_{m}")
                    nc.tensor.matmul(pj[:],
                                     lhsT=projW_bf[:, m * 128:(m + 1) * 128],
                                     rhs=ot[:], start=True, stop=True)
                    po = pr.tile([128, 512], BF16, tag="po",
                                 name=f"po_{cc}_{m}")
                    nc.vector.tensor_scalar(po[:], pj[:],
                                            projb_sb[:, m:m + 1], None,
                                            op0=mybir.AluOpType.add)
                    podma_pending.append((cc, m, po))

                def flush_podma():
                    while podma_pending:
                        cc, m, po = podma_pending.pop()
                        nc.sync.dma_start(
                            out_ext.ap()[m * 128:(m + 1) * 128,
                                         cc * 512:(cc + 1) * 512],
                            po[:])

                def emit_gk(qb):
                    kg = gp.tile([128, TOPK, BLK], BF16, tag="kg",
                                 name=f"kg_{qb}")
                    nc.gpsimd.ap_gather(kg[:], kT[:], kidx[:, qb:qb + 1],
                                        channels=128, num_elems=NB, d=BLK,
                                        num_idxs=TOPK)
                    kgs[qb] = kg

                def emit_vg(qb):
                    if qb not in state:
                        return
                    vg0 = gp.tile([128, TOPK, 66], BF16, tag="vg0",
                                  name=f"vg0_{qb}")
                    nc.gpsimd.ap_gather(vg0[:], v0[:], vidx0[:, qb:qb + 1],
                                        channels=128, num_elems=NB, d=66,
                                        num_idxs=TOPK)
                    vg1 = gp.tile([128, TOPK, 66], BF16, tag="vg1",
                                  name=f"vg1_{qb}")
                    nc.gpsimd.ap_gather(vg1[:], v1[:], vidx1[:, qb:qb + 1],
                                        channels=128, num_elems=NB, d=66,
                                        num_idxs=TOPK)
                    state[qb][1:3] = [vg0, vg1]

                def emit_smm(qb):
                    kg = kgs.pop(qb)
                    qcol = slice(qb * BLK, (qb + 1) * BLK)
                    etiles = [[None, None], [None, None]]
                    for half in range(2):
                        s0 = spp.tile([128, 1024], F32, tag="s",
                                      name=f"s0_{qb}_{half}")
                        s1 = spp.tile([128, 1024], F32, tag="s",
                                      name=f"s1_{qb}_{half}")
                        for jj in range(8):
                            j = half * 8 + jj
                            nc.tensor.matmul(s0[:, jj * 128:(jj + 1) * 128],
                                             lhsT=kg[0:64, j, :],
                                             rhs=qT[0:64, qcol],
                                             start=True, stop=True)
                            nc.tensor.matmul(s1[:, jj * 128:(jj + 1) * 128],
                                             lhsT=kg[64:128, j, :],
                                             rhs=qT[64:128, qcol],
                                             start=True, stop=True)
                        e0 = ep.tile([128, 1024], BF16, tag="e",
                                     name=f"e0_{qb}_{half}")
                        e1 = ep.tile([128, 1024], BF16, tag="e",
                                     name=f"e1_{qb}_{half}")
                        nc.scalar.activation(e0[:], s0[:],
                                             mybir.ActivationFunctionType.Exp,
                                             scale=SCALE)
                        nc.scalar.activation(e1[:], s1[:],
                                             mybir.ActivationFunctionType.Exp,
                                             scale=SCALE)
                        etiles[0][half] = e0
                        etiles[1][half] = e1
                    onorm = ob.tile([128, 2 * D], BF16, tag="onorm",
                                    name=f"on_{qb}")
                    state[qb] = [etiles, None, None, onorm]

                onorms = {}

                def emit_o(qb):
                    if qb not in state:
                        return
                    etiles, vg0, vg1, onorm = state[qb]
                    o_ps = opp.tile([128, 2, D + 1], F32, tag="o",
                                    name=f"o_{qb}")
                    for h in range(2):
                        vg = vg0 if h == 0 else vg1
                        for j in range(TOPK):
                            nc.tensor.matmul(
                                o_ps[:, h, :],
                                lhsT=etiles[h][j // 8][:, (j % 8) * 128:(j % 8 + 1) * 128],
                                rhs=vg[:, j, 0:D + 1],
                                start=(j == 0), stop=(j == TOPK - 1))
                    rec = ob.tile([128, 2], F32, tag="rec", name=f"r_{qb}")
                    nc.vector.reciprocal(rec[:], o_ps[:, :, D])
                    for h in range(2):
                        nc.vector.tensor_scalar(onorm[:, h * D:(h + 1) * D],
                                                o_ps[:, h, 0:D],
                                                rec[:, h:h + 1], None,
                                                op0=mybir.AluOpType.mult)
                    state.pop(qb)
                    onorms[qb] = onorm

                def emit_oT(qb):
                    # o -> o^T via SBUF->SBUF DMA transpose: costs no PE
                    # pair, no psum slot, and no DVE time
                    if qb not in onorms:
                        return
                    onorm = onorms.pop(qb)
                    cc = qb // CHQ
                    if qb % CHQ == 0:
                        chunk_ot[cc] = otp.tile([128, 512], BF16, tag="ot",
                                                name=f"ot_{cc}")
                    nc.sync.dma_start_transpose(
                        chunk_ot[cc][:, (qb % CHQ) * 128:(qb % CHQ + 1) * 128],
                        onorm[:])

                # proj m-tiles of chunk cc are spread across the following
                # chunk's iterations (2 per qb), keyed by the oT-complete qb
                proj_sched = {}

                def schedule_proj(cc):
                    base = CHQ * (cc + 1) + 2
                    for m in range(KC):
                        proj_sched.setdefault(base + m // 2, []).append((cc, m))

                # prologue: q + scores for the first two blocks flow while
                # the V phase computes, so the exp stream starts ~15us early
                emit_q(0)
                emit_q(1)
                emit_gk(0)
                emit_gk(1)
                emit_gk(2)
                emit_smm(0)
                emit_smm(1)
                for nch in range(NCH):
                    emit_v(nch)
                emit_vg(0)
                emit_vg(1)
                # steady state, per iteration qb: o(qb-1) -> oT -> proj a ->
                # k-gather(qb+2) -> scores(qb+1) -> proj b -> v-gather(qb+1);
                # scores go last because they self-throttle against the exp
                # drain (3 psum slots) and nothing may queue behind them
                for qb in range(1, NB):
                    cc = qb // CHQ
                    if qb % CHQ == 0 and cc + 1 < NCH:
                        emit_q(cc + 1)
                    emit_oT(qb - 2)
                    flush_podma()
                    emit_o(qb - 1)
                    pops = proj_sched.pop(qb, ())
                    if len(pops) > 0:
                        emit_proj_m(*pops[0])
                    if qb + 2 < NB:
                        emit_gk(qb + 2)
                    if qb + 1 < NB:
                        emit_smm(qb + 1)
                    if len(pops) > 1:
                        emit_proj_m(*pops[1])
                    if qb + 1 < NB:
                        emit_vg(qb + 1)
                    if qb >= 2 and (qb - 2) % CHQ == CHQ - 1:
                        schedule_proj((qb - 2) // CHQ)
                # drain
                emit_o(NB - 1)
                emit_oT(NB - 2)
                emit_oT(NB - 1)
                drain_idx = 0
                for qb in sorted(proj_sched):
                    for cm in proj_sched[qb]:
                        emit_proj_m(*cm, pool=((qp, "q"), (opp, "o"))[drain_idx % 2])
                        drain_idx += 1
                proj_sched.clear()
                for m in range(KC):
                    emit_proj_m(NCH - 1, m, pool=((qp, "q"), (opp, "o"))[drain_idx % 2])
                    drain_idx += 1
                flush_podma()

    nc.compile()
    return nc


def _prep_inputs(x, qkv_w, proj_w, proj_b):
    x = np.asarray(x, dtype=np.float32)
    qkv_w = np.asarray(qkv_w, dtype=np.float32)
    proj_w = np.asarray(proj_w, dtype=np.float32)
    proj_b = np.asarray(proj_b, dtype=np.float32)

    xT = np.ascontiguousarray(x[0].T)                         # [C, N]
    xbf = xT.astype(ml_dtypes.bfloat16)
    # exact f32 block sums: selection-equivalent to reference block means
    xm = np.ascontiguousarray(
        x[0].reshape(NB, BLK, C).sum(axis=1, dtype=np.float64).T
    ).astype(np.float32)                                      # [C, NB]
    ident64 = np.eye(64, dtype=np.float32)
    identb = np.eye(128, dtype=ml_dtypes.bfloat16)
    erep = (np.arange(128)[None, :] % 16 == np.arange(16)[:, None]).astype(np.float32)
    zero_b = np.zeros((128, KC), dtype=np.float32)
    in_maps = []
    for i in range(NCORES):
        h0 = HPC * i
        rows = []
        for part in range(3):                                 # q, k, v rows
            base = part * C + h0 * D
            rows.append(qkv_w[base:base + HPC * D, :])
        wqkv = np.concatenate(rows, axis=0)                   # [384, C]
        wqkvT = np.ascontiguousarray(wqkv.T)                  # [C, 384]
        cslice = slice(i * 2 * D, (i + 1) * 2 * D)
        in_maps.append({
            "xbf": xbf,
            "xm": xm,
            "wqksel": np.ascontiguousarray(wqkvT[:, 0:256]).astype(np.float32),
            "wqkvT": wqkvT.astype(ml_dtypes.bfloat16),
            "projWT": np.ascontiguousarray(proj_w[:, cslice].T).astype(ml_dtypes.bfloat16),
            "projb": (np.ascontiguousarray(proj_b.reshape(KC, 128).T)
                      if i == 0 else zero_b),
            "ident64": ident64,
            "identb": identb,
            "erep": erep,
        })
    return in_maps


def kernel(x, qkv_w, proj_w, proj_b, _trace=False):
    if "nc" not in _CACHE:
        _CACHE["nc"] = _build()
    nc = _CACHE["nc"]
    in_maps = _prep_inputs(x, qkv_w, proj_w, proj_b)
    res = run_bass_kernel_spmd(nc, in_maps, core_ids=list(range(NCORES)),
                               trace=_trace)
    outT = res.results[0]["out"].astype(np.float32)
    for i in range(1, NCORES):
        outT += res.results[i]["out"].astype(np.float32)
    out = np.ascontiguousarray(outT.T).reshape(1, N, C).astype(np.float32)
    if _trace:
        _CACHE["last_exec_time_ns"] = res.exec_time_ns
        _CACHE["last_results"] = res
    return out
